# revision 10
# baseline (speedup 1.0000x reference)
"""Trainium2 Bass kernel for per-image masked-softmax entropy (EntropyLoss).

Math (per (n, c) segment, over the HW=512*512 elements x of heatmap[n, c]):
    mask  = x > 0
    softmax over the masked elements, entropy in bits, summed over c and
    divided by the total positive count of image n.

The entropy of a masked softmax is invariant to the stabilizing shift m, so
we may use m = 0 (randn inputs keep exp(x) <= ~e^6, no overflow):
    S_c   = sum_{x>0} exp(x)
    U_c   = sum_{x>0} x * exp(x)
    ent_c = (log S_c - U_c / S_c) / ln2          [bits]
    out_n = sum_c ent_c / sum_c count_c

Device work per segment item [128, width] (bf16 x, cast during SWDGE DMA):
    r  = relu(x)                 (DVE tensor_scalar, 4x bf16)
    a  = exp(r)                  (ACT, fused accum -> S'_c = S_c + #nonpos)
    w  = a * r                   (DVE tensor_tensor, 2x bf16)
    mk = x > 0                   (DVE tensor_scalar, 4x bf16)
    U_c, count_c                 (PE: one-hot stationary weights route each
                                  segment's column sums into PSUM row c of a
                                  single [20, 512] accumulator; one final
                                  tensor_reduce folds 512 -> 1 for all rows)
S_c is recovered on the host as S'_c - (HW - count_c) since exp(0) = 1 for
every non-positive element. Final log/divide runs on host in float64.

Tiles are allocated ONCE and round-robined manually: every pool.tile() call
creates a distinct tile object and the TileContext teardown tail scales with
object/semaphore count (measured ~1.6us shorter with 44 objects vs 114).

Schedule: half-width items for the first and last TAPER segments (fast
pipeline fill and short drain), full segments in the middle.
"""

import os

import numpy as np

N, C, H, W = 8, 20, 512, 512
HW = H * W
P = 128
F = HW // P  # 2048
NCORES = 8
LN2 = 0.6931471805599453

DATA_BUFS = int(os.environ.get("ENTROPY_DATA_BUFS", "8"))
WARM_MM = int(os.environ.get("ENTROPY_WARM_MM", "24"))
TAPER_HEAD = int(os.environ.get("ENTROPY_TAPER_HEAD", "2"))
TAPER_TAIL = int(os.environ.get("ENTROPY_TAPER_TAIL", "2"))

_CACHE = {}

NSPLIT = TAPER_HEAD + TAPER_TAIL
SCOLS = C + NSPLIT


def _split_segments():
    return list(range(TAPER_HEAD)) + list(range(C - TAPER_TAIL, C))


def _schedule():
    """Work items: list of (c, lo, width, scol)."""
    split = _split_segments()
    items = []
    extra = C
    for c in range(C):
        if c in split:
            items.append((c, 0, F // 2, c))
            items.append((c, F // 2, F // 2, extra))
            extra += 1
        else:
            items.append((c, 0, F, c))
    assert extra == SCOLS
    return items


def _build_program():
    import concourse.bacc as bacc
    import concourse.mybir as mybir
    import concourse.tile as tile

    dt = mybir.dt
    Alu = mybir.AluOpType
    Act = mybir.ActivationFunctionType

    nc = bacc.Bacc(None, target_bir_lowering=False, debug=False)

    x_dram = nc.dram_tensor("x", [C, P, F], dt.float32, kind="ExternalInput")
    s_dram = nc.dram_tensor("s_out", [P, SCOLS], dt.float32, kind="ExternalOutput")
    r_dram = nc.dram_tensor("red_out", [C, 2], dt.float32, kind="ExternalOutput")

    items = _schedule()
    nmm = sum(w // 512 for _, _, w, _ in items)

    with tile.TileContext(nc) as tc:
        with (
            tc.tile_pool(name="const", bufs=1) as constp,
            tc.tile_pool(name="res", bufs=1) as resp,
            tc.tile_pool(name="data", bufs=1) as datap,
            tc.tile_pool(name="psum", bufs=1, space="PSUM") as psump,
        ):
            # Sliding-window one-hot weights: oh[:, 20 - c : 40 - c] is a
            # [128, 20] matrix whose only nonzero column (all ones) is c.
            oh = constp.tile([P, 2 * C], dt.bfloat16)
            nc.gpsimd.memset(oh[:], 0.0)
            nc.gpsimd.memset(oh[:, C : C + 1], 1.0)

            s_res = resp.tile([P, SCOLS], dt.float32)
            red = resp.tile([C, 2], dt.float32)

            u_psum = psump.tile([C, 512], dt.float32)
            c_psum = psump.tile([C, 512], dt.float32)

            # PE warmup: dummy matmuls during the DMA fill phase keep the
            # PE busy so HAM upclocks it to 2.4 GHz before real work lands.
            if WARM_MM:
                warm = constp.tile([P, 512], dt.bfloat16)
                nc.gpsimd.memset(warm[:], 0.0)
                w_psum = psump.tile([C, 512], dt.float32)
                for i in range(WARM_MM):
                    nc.tensor.matmul(
                        w_psum[:], oh[:, 0:C], warm[:],
                        start=(i == 0), stop=(i == WARM_MM - 1),
                    )

            # Long-lived round-robin buffers (see module docstring).
            nb = DATA_BUFS
            def mktiles(pfx):
                return [
                    datap.tile(
                        [P, F], dt.bfloat16, tag=f"{pfx}{i}", name=f"{pfx}{i}"
                    )
                    for i in range(nb)
                ]

            xs = mktiles("x")
            rs = mktiles("r")
            as_ = mktiles("a")
            ws = mktiles("w")
            mks = mktiles("m")

            mm = 0  # chunk-matmul index, for start/stop flags
            for idx, (c, lo, width, scol) in enumerate(items):
                b = idx % nb
                x_t = xs[b][:, :width]
                r_t = rs[b][:, :width]
                a_t = as_[b][:, :width]
                w_t = ws[b][:, :width]
                mk_t = mks[b][:, :width]

                # SWDGE DMA casts fp32 -> bf16 on the fly.
                nc.gpsimd.dma_start(x_t, x_dram[c, :, lo : lo + width])

                nc.vector.tensor_scalar(r_t, x_t, 0.0, None, Alu.max)
                nc.scalar.activation(
                    a_t, r_t, Act.Exp,
                    accum_out=s_res[:, scol : scol + 1],
                )
                # mk before the mult: the mult waits on ACT's exp, and DVE's
                # queue is strict FIFO — issuing mk first keeps DVE busy
                # during the exp latency instead of stalling behind the mult.
                nc.vector.tensor_scalar(mk_t, x_t, 0.0, None, Alu.is_gt)
                nc.vector.tensor_tensor(w_t, a_t, r_t, Alu.mult)

                lhsT = oh[:, C - c : 2 * C - c]
                nj = width // 512
                for j in range(nj):
                    sl = slice(j * 512, (j + 1) * 512)
                    nc.tensor.matmul(
                        u_psum[:], lhsT, w_t[:, sl],
                        start=(mm + j == 0), stop=(mm + j == nmm - 1),
                    )
                for j in range(nj):
                    sl = slice(j * 512, (j + 1) * 512)
                    nc.tensor.matmul(
                        c_psum[:], lhsT, mk_t[:, sl],
                        start=(mm + j == 0), stop=(mm + j == nmm - 1),
                    )
                mm += nj

            nc.vector.tensor_reduce(
                red[:, 0:1], u_psum[:], mybir.AxisListType.X, Alu.add
            )
            nc.vector.tensor_reduce(
                red[:, 1:2], c_psum[:], mybir.AxisListType.X, Alu.add
            )
            nc.sync.dma_start(s_dram[:], s_res[:])
            nc.sync.dma_start(r_dram[:], red[:])

    nc.compile()
    return nc


def _get_program():
    if "nc" not in _CACHE:
        _CACHE["nc"] = _build_program()
    return _CACHE["nc"]


def _run(heatmap: np.ndarray, trace: bool = False):
    from concourse.bass_utils import run_bass_kernel_spmd

    nc = _get_program()
    in_maps = [
        {"x": np.ascontiguousarray(heatmap[i].reshape(C, P, F), dtype=np.float32)}
        for i in range(NCORES)
    ]
    return run_bass_kernel_spmd(nc, in_maps, list(range(NCORES)), trace=trace)


def _finalize(results) -> np.ndarray:
    """Host epilogue: a few scalars per core -> entropy[n] in float64."""
    out = np.zeros(N, dtype=np.float64)
    split = _split_segments()
    for n in range(NCORES):
        r = results[n]
        s_full = r["s_out"].astype(np.float64).sum(axis=0)   # [SCOLS]
        s_prime = s_full[:C].copy()
        for k, c in enumerate(split):
            s_prime[c] += s_full[C + k]
        red = r["red_out"].astype(np.float64)                # [C, 2]
        u = red[:, 0]
        cnt = red[:, 1]
        s = s_prime - (HW - cnt)                             # masked sum exp
        ent = np.zeros(C, dtype=np.float64)
        ok = s > 0
        ent[ok] = (np.log(s[ok]) - u[ok] / s[ok]) / LN2
        out[n] = ent.sum() / cnt.sum()
    return out.astype(np.float32)


def kernel(heatmap: np.ndarray) -> np.ndarray:
    heatmap = np.asarray(heatmap, dtype=np.float32)
    assert heatmap.shape == (N, C, H, W), heatmap.shape
    res = _run(heatmap, trace=False)
    return _finalize(res.results)
